# revision 8
# baseline (speedup 1.0000x reference)
"""ChebyKAN layer kernel for Trainium2 (8 NeuronCores) — v2.

Computes y[b,o] = sum_{i,d} T_d(tanh(x)[b,i]) * C[i,o,d] as a
(8192 x 8320) @ (8320 x 1024) f16 matmul after building product-basis
features on-chip (see prep_inputs for the basis change; the constant
feature is folded into a 128-row ones-tile block).

Sharding: 4-way batch x 2-way output columns (core c -> batch group
c//2, output group c%2). No collectives.

Key layout decisions (all HW-probed, see session notes):
- Weights are the STATIONARY matmul operand; features stream as moving
  data. Each stationary [128i x 128o] slice serves 2 consecutive N=512
  matmuls (batch phase = 1024), which amortizes the otherwise
  serialized LDWEIGHTS (~280 -> ~225 ns/matmul measured). Explicit
  nc.tensor.ldweights is WORSE (double-emits the load).
- Output is produced transposed ([O_SH, B_SH] per core) and fixed on
  host, so no on-chip transposes are needed.
- Whole pipeline in fp16 (e5m10): x is transported as f16, the f16
  feature-chain intermediates feed the PE directly (no bf16 copies),
  f16 weights carry 2 extra mantissa bits over bf16. PSUM is f32.
  absmax-rel vs the f32 reference: 1.7e-3.
- x is host-packed so each phase loads one [128, 8192] f16 tile in a
  single DMA, and tanh runs once per phase as a single wide ACT op:
  per-tile DMA-gated producer ops measurably starve the PE (~3.3 us
  per gated tile-op), so the only x-gated op is 1 wide tanh per phase.
- PSUM evacuation runs on DVE (gpsimd cannot access PSUM) and the y
  store DMAs issue from the gpsimd ring, keeping the ACT stream (tanh/
  Square chain) clear of phase-boundary drains. The evac block for
  phase N is emitted after phase N+1's first feature-chain group (with
  the psum alloc + ones matmuls deferred past it, keeping pool
  generations in emission order), so the in-order DVE stream never
  queues next-phase chain work behind psum drains (+2% measured).
"""

from contextlib import ExitStack

import numpy as np

import concourse.bacc as bacc
import concourse.mybir as mybir
import concourse.tile as tile
from concourse.bass_utils import run_bass_kernel_spmd

P = 128
B_FULL, I_DIM, O_FULL, DEG = 8192, 1024, 1024, 8
N_CORES = 8
BG, OG = 4, 2  # core grid: batch groups x output groups
B_SH = B_FULL // BG  # 2048 batch rows per core
O_SH = O_FULL // OG  # 512 output cols per core
PH = 1024  # batch-phase width
N_PH = B_SH // PH  # 2 phases
NBH = PH // 512  # 2 moving-chunks per stationary
NIT = I_DIM // P  # 8 i-tiles
KT = 1 + DEG * NIT  # 65 contraction tiles (1 ones + 64 features)
NOT = O_SH // P  # 4 output tiles
WS = 64.0  # host-side weight scale (descaled in psum evac)
F32 = mybir.dt.float32
F16 = mybir.dt.float16
MULT = mybir.AluOpType.mult
ADD = mybir.AluOpType.add
TANH = mybir.ActivationFunctionType.Tanh
SQUARE = mybir.ActivationFunctionType.Square
COPY = mybir.ActivationFunctionType.Copy

USE_LDW = False  # probe: explicit ldweights double-emits the load (slower)

_NC_CACHE = []


def _build_ir(repeat=1, loop_iters=None, variant="full"):
    nc = bacc.Bacc(
        "TRN2", target_bir_lowering=False, debug=False, enable_asserts=False
    )
    xP = nc.dram_tensor("xP", [N_PH * P, NIT * PH], F16, kind="ExternalInput").ap()
    wv = nc.dram_tensor("wv", [KT * P, O_SH], F16, kind="ExternalInput").ap()
    yT = nc.dram_tensor("yT", [O_SH, B_SH], F32, kind="ExternalOutput").ap()

    with ExitStack() as ctx:
        tc = ctx.enter_context(tile.TileContext(nc))
        wpool = ctx.enter_context(tc.tile_pool(name="w", bufs=1))
        opool = ctx.enter_context(tc.tile_pool(name="o", bufs=1))
        xpool = ctx.enter_context(tc.tile_pool(name="x", bufs=2))
        fb = {"full4": 4, "full3": 3}.get(variant, 2)
        fpool = ctx.enter_context(tc.tile_pool(name="f", bufs=fb))
        cpool = ctx.enter_context(tc.tile_pool(name="c", bufs=fb))
        tpool = ctx.enter_context(tc.tile_pool(name="t1", bufs=2))
        ypool = ctx.enter_context(tc.tile_pool(name="yp", bufs=8))
        pspool = ctx.enter_context(tc.tile_pool(name="ps", bufs=8, space="PSUM"))

        ones = opool.tile([P, 512], F16, tag="ones")
        nc.vector.memset(ones[:], 1.0)

        wt = [
            wpool.tile([P, O_SH], F16, tag=f"w{k}", name=f"w{k}") for k in range(KT)
        ]
        nc.sync.dma_start(out=wt[0][:], in_=wv[0:P, :])
        if loop_iters is not None:
            # timing variants: weights fully resident before the hw loop
            for k in range(1, KT):
                nc.sync.dma_start(out=wt[k][:], in_=wv[k * P : (k + 1) * P, :])

        fixed_feat = None
        if variant in ("pe", "mm", "nochain", "tanhonly", "halfchain", "tanh1",
                       "chainload"):
            pfpool = ctx.enter_context(tc.tile_pool(name="pf", bufs=1))
            fixed_feat = [
                pfpool.tile([P, PH], F16, tag=f"pf{d}", name=f"pf{d}")
                for d in range(1, DEG + 1)
            ]
            for t in fixed_feat:
                nc.vector.memset(t[:], 0.01)
        xz = None
        if variant == "nox":
            xzpool = ctx.enter_context(tc.tile_pool(name="xz", bufs=1))
            xz = xzpool.tile([P, PH], F16, tag="xz")
            nc.vector.memset(xz[:], 0.3)

        def emit_body(rep):
            pending_evac = [None]

            def flush_evac():
                if pending_evac[0] is not None:
                    pending_evac[0]()
                    pending_evac[0] = None

            for ph in range(N_PH):
                b0 = ph * PH
                xbig = None
                t1big = None
                if variant not in ("pe", "mm", "nox"):
                    xbig = xpool.tile([P, NIT * PH], F16, tag="xr")
                    nc.sync.dma_start(
                        out=xbig[:], in_=xP[ph * P : (ph + 1) * P, :]
                    )
                    # one wide tanh per phase: the only x-DMA-gated op; all
                    # downstream per-it work gates on this early single op
                    t1big = tpool.tile([P, NIT * PH], F16, tag="t1big")
                    nc.scalar.activation(t1big[:], xbig[:], TANH)
                def alloc_and_ones():
                    # psum alloc + ones k-tile; deferred to after flush_evac so
                    # pool generations stay in emission order
                    ps = [
                        [
                            pspool.tile([P, 512], F32, tag="ps", name="ps")
                            for _ in range(NBH)
                        ]
                        for _ in range(NOT)
                    ]
                    for ot in range(NOT):
                        s = wt[0][:, ot * P : (ot + 1) * P]
                        for bh in range(NBH):
                            nc.tensor.matmul(
                                ps[ot][bh][:], s, ones[:], start=True, stop=False
                            )
                    return ps

                psums = None
                for it in range(NIT):
                    if variant in ("pe", "mm", "nochain", "tanhonly", "halfchain", "tanh1"):
                        if psums is None:
                            flush_evac()
                            psums = alloc_and_ones()
                        if variant in ("tanhonly", "halfchain") or (
                            variant == "tanh1" and it == 0
                        ):
                            xr = xbig[:, it * PH : (it + 1) * PH]
                            f = [None] + list(fixed_feat)
                            ftmp = fpool.tile([P, PH], F16, tag="f1", name="f1")
                            nc.scalar.activation(ftmp[:], xr[:], TANH)
                            f[1] = ftmp
                            if variant == "halfchain":
                                f2t = fpool.tile([P, PH], F16, tag="f2", name="f2")
                                nc.scalar.activation(f2t[:], f[1][:], SQUARE)
                                f[2] = f2t
                                t2 = cpool.tile([P, PH], F16, tag="t2")
                                nc.vector.tensor_scalar(t2[:], f[2][:], 2.0, -1.0, MULT, ADD)
                                f3t = fpool.tile([P, PH], F16, tag="f3", name="f3")
                                nc.vector.tensor_tensor(f3t[:], f[1][:], t2[:], MULT)
                                f[3] = f3t
                                f4t = fpool.tile([P, PH], F16, tag="f4", name="f4")
                                nc.vector.tensor_tensor(f4t[:], t2[:], t2[:], MULT)
                                f[4] = f4t
                            for d in range(1, DEG + 1):
                                k = 1 + (d - 1) * NIT + it
                                last = it == NIT - 1 and d == DEG
                                for ot in range(NOT):
                                    sw = wt[k][:, ot * P : (ot + 1) * P]
                                    for bh in range(NBH):
                                        nc.tensor.matmul(
                                            psums[ot][bh][:],
                                            sw,
                                            f[d][:, bh * 512 : (bh + 1) * 512],
                                            start=False,
                                            stop=last,
                                        )
                            continue
                        f = [None] + fixed_feat
                        for d in range(1, DEG + 1):
                            k = 1 + (d - 1) * NIT + it
                            last = it == NIT - 1 and d == DEG
                            for ot in range(NOT):
                                s = wt[k][:, ot * P : (ot + 1) * P]
                                for bh in range(NBH):
                                    nc.tensor.matmul(
                                        psums[ot][bh][:],
                                        s,
                                        f[d][:, bh * 512 : (bh + 1) * 512],
                                        start=False,
                                        stop=last,
                                    )
                        continue
                    xr = xz if variant == "nox" else None
                    if ph == 0 and rep == 0 and loop_iters is None:
                        for d in range(1, DEG + 1):
                            k = 1 + (d - 1) * NIT + it
                            nc.sync.dma_start(
                                out=wt[k][:], in_=wv[k * P : (k + 1) * P, :]
                            )
                    # f16 feature chain; Chebyshev-product basis:
                    # f1=t, f2=t^2, t2=2f2-1, f3=t*t2, f4=t2^2, t4=2f4-1,
                    # f5=t*t4, f6=t2*t4, f7=t4*f3, f8=t4^2
                    f = [None] * (DEG + 1)

                    def feat(d):
                        f[d] = fpool.tile(
                            [P, PH], F16, tag=f"f{d}", name=f"f{d}"
                        )
                        return f[d]

                    f[1] = t1big[:, it * PH : (it + 1) * PH] if t1big is not None else None
                    if f[1] is None:
                        nc.scalar.activation(feat(1)[:], xr[:], TANH)
                    nc.scalar.activation(feat(2)[:], f[1][:], SQUARE)
                    t2 = cpool.tile([P, PH], F16, tag="t2")
                    nc.vector.tensor_scalar(t2[:], f[2][:], 2.0, -1.0, MULT, ADD)
                    nc.vector.tensor_tensor(feat(3)[:], f[1][:], t2[:], MULT)
                    nc.vector.tensor_tensor(feat(4)[:], t2[:], t2[:], MULT)
                    t4 = cpool.tile([P, PH], F16, tag="t4")
                    nc.scalar.activation(t4[:], f[4][:], COPY, bias=-1.0, scale=2.0)
                    nc.vector.tensor_tensor(feat(5)[:], f[1][:], t4[:], MULT)
                    nc.vector.tensor_tensor(feat(6)[:], t2[:], t4[:], MULT)
                    nc.vector.tensor_tensor(feat(7)[:], t4[:], f[3][:], MULT)
                    nc.vector.tensor_tensor(feat(8)[:], t4[:], t4[:], MULT)
                    if it == 0:
                        flush_evac()
                        psums = alloc_and_ones()

                    for d in range(1, DEG + 1):
                        k = 1 + (d - 1) * NIT + it
                        last = it == NIT - 1 and d == DEG
                        fd = (
                            fixed_feat[d - 1]
                            if variant == "chainload"
                            else f[d]
                        )
                        for ot in range(NOT):
                            s = wt[k][:, ot * P : (ot + 1) * P]
                            if USE_LDW:
                                nc.tensor.ldweights(s)
                            for bh in range(NBH):
                                nc.tensor.matmul(
                                    psums[ot][bh][:],
                                    s,
                                    fd[:, bh * 512 : (bh + 1) * 512],
                                    start=False,
                                    stop=last,
                                )
                def make_evac(psums, b0):
                    def emit():
                        if variant == "mm":
                            return
                        for ot in range(NOT):
                            for bh in range(NBH):
                                ysb = ypool.tile([P, 512], F32, tag="ysb")
                                # evac on DVE (gpsimd cannot touch PSUM)
                                nc.vector.tensor_scalar(
                                    ysb[:], psums[ot][bh][:], 1.0 / WS, 0.0,
                                    MULT, ADD,
                                )
                                if variant != "noy":
                                    nc.gpsimd.dma_start(
                                        out=yT[
                                            ot * P : (ot + 1) * P,
                                            b0 + bh * 512 : b0 + (bh + 1) * 512,
                                        ],
                                        in_=ysb[:],
                                    )
                    return emit

                flush_evac()  # no-op except variants that skip the it-loop
                pending_evac[0] = make_evac(psums, b0)
            flush_evac()

        if loop_iters is not None:
            with tc.For_i(0, loop_iters, 1):
                emit_body(0)
        else:
            for rep in range(repeat):
                emit_body(rep)
    nc.compile()
    return nc


def get_nc():
    if not _NC_CACHE:
        _NC_CACHE.append(_build_ir())
    return _NC_CACHE[0]


def prep_inputs(x, cheby_coeffs):
    """Host-side shard prep: returns per-core input maps."""
    x = np.asarray(x, dtype=np.float32)
    c = np.asarray(cheby_coeffs, dtype=np.float64)
    w = [c[:, :, d] for d in range(DEG + 1)]
    v = [
        w[0] - w[2] - w[4] + w[6] - w[8],
        w[1] - w[3] + w[5] - w[7],
        2.0 * (w[2] - w[6]),
        2.0 * (w[3] - w[5]),
        2.0 * w[4],
        2.0 * (w[5] - w[7]),
        2.0 * w[6],
        4.0 * w[7],
        2.0 * w[8],
    ]
    v0r = v[0].reshape(NIT, P, O_FULL).sum(axis=0)  # (128, 1024)
    wv_full = np.concatenate([v0r] + v[1:], axis=0) * WS  # (8320, 1024)
    wv_f16 = wv_full.astype(np.float16)
    xt_full = x.T.astype(np.float16)  # (1024, 8192)

    in_maps = []
    for core in range(N_CORES):
        bg, og = core // OG, core % OG
        in_maps.append(
            {
                # pack so each partition's per-phase x is contiguous:
                # xP[ph*128+p, it*PH+b] = xT[it*128+p, ph*PH+b]
                "xP": np.ascontiguousarray(
                    xt_full[:, bg * B_SH : (bg + 1) * B_SH]
                    .reshape(NIT, P, N_PH, PH)
                    .transpose(2, 1, 0, 3)
                    .reshape(N_PH * P, NIT * PH)
                ),
                "wv": np.ascontiguousarray(
                    wv_f16[:, og * O_SH : (og + 1) * O_SH]
                ),
            }
        )
    return in_maps


def assemble_output(results):
    y_full = np.empty((B_FULL, O_FULL), dtype=np.float32)
    for core in range(N_CORES):
        bg, og = core // OG, core % OG
        y_full[bg * B_SH : (bg + 1) * B_SH, og * O_SH : (og + 1) * O_SH] = (
            np.asarray(results[core]["yT"], dtype=np.float32).T
        )
    return y_full


def kernel(x, cheby_coeffs):
    nc = get_nc()
    in_maps = prep_inputs(x, cheby_coeffs)
    res = run_bass_kernel_spmd(nc, in_maps, list(range(N_CORES)))
    return assemble_output(res.results)



# revision 9
# speedup vs baseline: 1.0049x; 1.0049x over previous
"""ChebyKAN layer kernel for Trainium2 (8 NeuronCores) — v2.

Computes y[b,o] = sum_{i,d} T_d(tanh(x)[b,i]) * C[i,o,d] as a
(8192 x 8320) @ (8320 x 1024) f16 matmul after building product-basis
features on-chip (see prep_inputs for the basis change; the constant
feature is folded into a 128-row ones-tile block).

Sharding: 4-way batch x 2-way output columns (core c -> batch group
c//2, output group c%2). No collectives.

Key layout decisions (all HW-probed, see session notes):
- Weights are the STATIONARY matmul operand; features stream as moving
  data. Each stationary [128i x 128o] slice serves 2 consecutive N=512
  matmuls (batch phase = 1024), which amortizes the otherwise
  serialized LDWEIGHTS (~280 -> ~225 ns/matmul measured). Explicit
  nc.tensor.ldweights is WORSE (double-emits the load).
- Output is produced transposed ([O_SH, B_SH] per core) and fixed on
  host, so no on-chip transposes are needed.
- Whole pipeline in fp16 (e5m10): x is transported as f16, the f16
  feature-chain intermediates feed the PE directly (no bf16 copies),
  f16 weights carry 2 extra mantissa bits over bf16. PSUM is f32.
  absmax-rel vs the f32 reference: 1.7e-3.
- x is host-packed so each phase loads one [128, 8192] f16 tile in a
  single DMA, and tanh runs once per phase as a single wide ACT op:
  per-tile DMA-gated producer ops measurably starve the PE (~3.3 us
  per gated tile-op), so the only x-gated op is 1 wide tanh per phase.
- PSUM evacuation runs on DVE (gpsimd cannot access PSUM) and the y
  store DMAs issue from the gpsimd ring, keeping the ACT stream (tanh/
  Square chain) clear of phase-boundary drains. The evac block for
  phase N is emitted after phase N+1's first feature-chain group (with
  the psum alloc + ones matmuls deferred past it, keeping pool
  generations in emission order), so the in-order DVE stream never
  queues next-phase chain work behind psum drains (+2% measured).
"""

from contextlib import ExitStack

import numpy as np

import concourse.bacc as bacc
import concourse.mybir as mybir
import concourse.tile as tile
from concourse.bass_utils import run_bass_kernel_spmd

P = 128
B_FULL, I_DIM, O_FULL, DEG = 8192, 1024, 1024, 8
N_CORES = 8
BG, OG = 4, 2  # core grid: batch groups x output groups
B_SH = B_FULL // BG  # 2048 batch rows per core
O_SH = O_FULL // OG  # 512 output cols per core
PH = 1024  # batch-phase width
N_PH = B_SH // PH  # 2 phases
NBH = PH // 512  # 2 moving-chunks per stationary
NIT = I_DIM // P  # 8 i-tiles
KT = 1 + DEG * NIT  # 65 contraction tiles (1 ones + 64 features)
NOT = O_SH // P  # 4 output tiles
WS = 64.0  # host-side weight scale (descaled in psum evac)
F32 = mybir.dt.float32
F16 = mybir.dt.float16
MULT = mybir.AluOpType.mult
ADD = mybir.AluOpType.add
TANH = mybir.ActivationFunctionType.Tanh
SQUARE = mybir.ActivationFunctionType.Square
COPY = mybir.ActivationFunctionType.Copy

USE_LDW = False  # probe: explicit ldweights double-emits the load (slower)

_NC_CACHE = []


def _build_ir(repeat=1, loop_iters=None, variant="full"):
    nc = bacc.Bacc(
        "TRN2", target_bir_lowering=False, debug=False, enable_asserts=False
    )
    xP = nc.dram_tensor("xP", [N_PH * P, NIT * PH], F16, kind="ExternalInput").ap()
    wv = nc.dram_tensor("wv", [KT * P, O_SH], F16, kind="ExternalInput").ap()
    yT = nc.dram_tensor("yT", [O_SH, B_SH], F32, kind="ExternalOutput").ap()

    with ExitStack() as ctx:
        tc = ctx.enter_context(tile.TileContext(nc))
        wpool = ctx.enter_context(tc.tile_pool(name="w", bufs=1))
        opool = ctx.enter_context(tc.tile_pool(name="o", bufs=1))
        xpool = ctx.enter_context(tc.tile_pool(name="x", bufs=2))
        fb = {"full4": 4, "full3": 3}.get(variant, 2)
        fpool = ctx.enter_context(tc.tile_pool(name="f", bufs=fb))
        cpool = ctx.enter_context(tc.tile_pool(name="c", bufs=fb))
        tpool = ctx.enter_context(tc.tile_pool(name="t1", bufs=2))
        ypool = ctx.enter_context(tc.tile_pool(name="yp", bufs=8))
        pspool = ctx.enter_context(tc.tile_pool(name="ps", bufs=8, space="PSUM"))

        ones = opool.tile([P, 512], F16, tag="ones")
        nc.vector.memset(ones[:], 1.0)

        wt = [
            wpool.tile([P, O_SH], F16, tag=f"w{k}", name=f"w{k}") for k in range(KT)
        ]
        nc.sync.dma_start(out=wt[0][:], in_=wv[0:P, :])
        if loop_iters is not None:
            # timing variants: weights fully resident before the hw loop
            for k in range(1, KT):
                nc.sync.dma_start(out=wt[k][:], in_=wv[k * P : (k + 1) * P, :])

        fixed_feat = None
        if variant in ("pe", "mm", "nochain", "tanhonly", "halfchain", "tanh1",
                       "chainload"):
            pfpool = ctx.enter_context(tc.tile_pool(name="pf", bufs=1))
            fixed_feat = [
                pfpool.tile([P, PH], F16, tag=f"pf{d}", name=f"pf{d}")
                for d in range(1, DEG + 1)
            ]
            for t in fixed_feat:
                nc.vector.memset(t[:], 0.01)
        xz = None
        if variant == "nox":
            xzpool = ctx.enter_context(tc.tile_pool(name="xz", bufs=1))
            xz = xzpool.tile([P, PH], F16, tag="xz")
            nc.vector.memset(xz[:], 0.3)

        def emit_body(rep):
            pending_evac = [None]

            def flush_evac():
                if pending_evac[0] is not None:
                    pending_evac[0]()
                    pending_evac[0] = None

            for ph in range(N_PH):
                b0 = ph * PH
                xbig = None
                t1big = None
                if variant not in ("pe", "mm", "nox"):
                    xbig = xpool.tile([P, NIT * PH], F16, tag="xr")
                    nc.sync.dma_start(
                        out=xbig[:], in_=xP[ph * P : (ph + 1) * P, :]
                    )
                    # one wide tanh per phase: the only x-DMA-gated op; all
                    # downstream per-it work gates on this early single op
                    t1big = tpool.tile([P, NIT * PH], F16, tag="t1big")
                    nc.scalar.activation(t1big[:], xbig[:], TANH)
                def alloc_and_ones():
                    # psum alloc + ones k-tile; deferred to after flush_evac so
                    # pool generations stay in emission order
                    ps = [
                        [
                            pspool.tile([P, 512], F32, tag="ps", name="ps")
                            for _ in range(NBH)
                        ]
                        for _ in range(NOT)
                    ]
                    for ot in range(NOT):
                        s = wt[0][:, ot * P : (ot + 1) * P]
                        for bh in range(NBH):
                            nc.tensor.matmul(
                                ps[ot][bh][:], s, ones[:], start=True, stop=False
                            )
                    return ps

                psums = None
                for it in range(NIT):
                    if variant in ("pe", "mm", "nochain", "tanhonly", "halfchain", "tanh1"):
                        if psums is None:
                            flush_evac()
                            psums = alloc_and_ones()
                        if variant in ("tanhonly", "halfchain") or (
                            variant == "tanh1" and it == 0
                        ):
                            xr = xbig[:, it * PH : (it + 1) * PH]
                            f = [None] + list(fixed_feat)
                            ftmp = fpool.tile([P, PH], F16, tag="f1", name="f1")
                            nc.scalar.activation(ftmp[:], xr[:], TANH)
                            f[1] = ftmp
                            if variant == "halfchain":
                                f2t = fpool.tile([P, PH], F16, tag="f2", name="f2")
                                nc.scalar.activation(f2t[:], f[1][:], SQUARE)
                                f[2] = f2t
                                t2 = cpool.tile([P, PH], F16, tag="t2")
                                nc.vector.tensor_scalar(t2[:], f[2][:], 2.0, -1.0, MULT, ADD)
                                f3t = fpool.tile([P, PH], F16, tag="f3", name="f3")
                                nc.vector.tensor_tensor(f3t[:], f[1][:], t2[:], MULT)
                                f[3] = f3t
                                f4t = fpool.tile([P, PH], F16, tag="f4", name="f4")
                                nc.vector.tensor_tensor(f4t[:], t2[:], t2[:], MULT)
                                f[4] = f4t
                            for d in range(1, DEG + 1):
                                k = 1 + (d - 1) * NIT + it
                                last = it == NIT - 1 and d == DEG
                                for ot in range(NOT):
                                    sw = wt[k][:, ot * P : (ot + 1) * P]
                                    for bh in range(NBH):
                                        nc.tensor.matmul(
                                            psums[ot][bh][:],
                                            sw,
                                            f[d][:, bh * 512 : (bh + 1) * 512],
                                            start=False,
                                            stop=last,
                                        )
                            continue
                        f = [None] + fixed_feat
                        for d in range(1, DEG + 1):
                            k = 1 + (d - 1) * NIT + it
                            last = it == NIT - 1 and d == DEG
                            for ot in range(NOT):
                                s = wt[k][:, ot * P : (ot + 1) * P]
                                for bh in range(NBH):
                                    nc.tensor.matmul(
                                        psums[ot][bh][:],
                                        s,
                                        f[d][:, bh * 512 : (bh + 1) * 512],
                                        start=False,
                                        stop=last,
                                    )
                        continue
                    xr = xz if variant == "nox" else None
                    if ph == 0 and rep == 0 and loop_iters is None:
                        for d in range(1, DEG + 1):
                            k = 1 + (d - 1) * NIT + it
                            nc.sync.dma_start(
                                out=wt[k][:], in_=wv[k * P : (k + 1) * P, :]
                            )
                    # f16 feature chain; Chebyshev-product basis:
                    # f1=t, f2=t^2, t2=2f2-1, f3=t*t2, f4=t2^2, t4=2f4-1,
                    # f5=t*t4, f6=t2*t4, f7=t4*f3, f8=t4^2
                    f = [None] * (DEG + 1)

                    def feat(d):
                        f[d] = fpool.tile(
                            [P, PH], F16, tag=f"f{d}", name=f"f{d}"
                        )
                        return f[d]

                    f[1] = t1big[:, it * PH : (it + 1) * PH] if t1big is not None else None
                    if f[1] is None:
                        nc.scalar.activation(feat(1)[:], xr[:], TANH)
                    nc.scalar.activation(feat(2)[:], f[1][:], SQUARE)
                    t2 = cpool.tile([P, PH], F16, tag="t2")
                    nc.vector.tensor_scalar(t2[:], f[2][:], 2.0, -1.0, MULT, ADD)
                    nc.vector.tensor_tensor(feat(3)[:], f[1][:], t2[:], MULT)
                    nc.vector.tensor_tensor(feat(4)[:], t2[:], t2[:], MULT)
                    t4 = cpool.tile([P, PH], F16, tag="t4")
                    nc.scalar.activation(t4[:], f[4][:], COPY, bias=-1.0, scale=2.0)
                    e56 = nc.gpsimd if variant == "fullb" else nc.vector
                    e56.tensor_tensor(feat(5)[:], f[1][:], t4[:], MULT)
                    e56.tensor_tensor(feat(6)[:], t2[:], t4[:], MULT)
                    nc.vector.tensor_tensor(feat(7)[:], t4[:], f[3][:], MULT)
                    nc.vector.tensor_tensor(feat(8)[:], t4[:], t4[:], MULT)
                    if it == 0:
                        flush_evac()
                        psums = alloc_and_ones()

                    for d in range(1, DEG + 1):
                        k = 1 + (d - 1) * NIT + it
                        last = it == NIT - 1 and d == DEG
                        fd = (
                            fixed_feat[d - 1]
                            if variant == "chainload"
                            else f[d]
                        )
                        for ot in range(NOT):
                            s = wt[k][:, ot * P : (ot + 1) * P]
                            if USE_LDW:
                                nc.tensor.ldweights(s)
                            for bh in range(NBH):
                                nc.tensor.matmul(
                                    psums[ot][bh][:],
                                    s,
                                    fd[:, bh * 512 : (bh + 1) * 512],
                                    start=False,
                                    stop=last,
                                )
                def make_evac(psums, b0):
                    def emit():
                        if variant == "mm":
                            return
                        for ot in range(NOT):
                            for bh in range(NBH):
                                ysb = ypool.tile([P, 512], F32, tag="ysb")
                                if variant == "fullb":
                                    # evac on ACT: keeps the DVE queue free
                                    # for chain ops (JIT producer regime)
                                    nc.scalar.activation(
                                        ysb[:], psums[ot][bh][:], COPY,
                                        scale=1.0 / WS,
                                    )
                                else:
                                    # evac on DVE (gpsimd cannot touch PSUM)
                                    nc.vector.tensor_scalar(
                                        ysb[:], psums[ot][bh][:], 1.0 / WS, 0.0,
                                        MULT, ADD,
                                    )
                                if variant != "noy":
                                    nc.gpsimd.dma_start(
                                        out=yT[
                                            ot * P : (ot + 1) * P,
                                            b0 + bh * 512 : b0 + (bh + 1) * 512,
                                        ],
                                        in_=ysb[:],
                                    )
                    return emit

                flush_evac()  # no-op except variants that skip the it-loop
                pending_evac[0] = make_evac(psums, b0)
            flush_evac()

        if loop_iters is not None:
            with tc.For_i(0, loop_iters, 1):
                emit_body(0)
        else:
            for rep in range(repeat):
                emit_body(rep)
    nc.compile()
    return nc


def get_nc():
    if not _NC_CACHE:
        _NC_CACHE.append(_build_ir())
    return _NC_CACHE[0]


def prep_inputs(x, cheby_coeffs):
    """Host-side shard prep: returns per-core input maps."""
    x = np.asarray(x, dtype=np.float32)
    c = np.asarray(cheby_coeffs, dtype=np.float64)
    w = [c[:, :, d] for d in range(DEG + 1)]
    v = [
        w[0] - w[2] - w[4] + w[6] - w[8],
        w[1] - w[3] + w[5] - w[7],
        2.0 * (w[2] - w[6]),
        2.0 * (w[3] - w[5]),
        2.0 * w[4],
        2.0 * (w[5] - w[7]),
        2.0 * w[6],
        4.0 * w[7],
        2.0 * w[8],
    ]
    v0r = v[0].reshape(NIT, P, O_FULL).sum(axis=0)  # (128, 1024)
    wv_full = np.concatenate([v0r] + v[1:], axis=0) * WS  # (8320, 1024)
    wv_f16 = wv_full.astype(np.float16)
    xt_full = x.T.astype(np.float16)  # (1024, 8192)

    in_maps = []
    for core in range(N_CORES):
        bg, og = core // OG, core % OG
        in_maps.append(
            {
                # pack so each partition's per-phase x is contiguous:
                # xP[ph*128+p, it*PH+b] = xT[it*128+p, ph*PH+b]
                "xP": np.ascontiguousarray(
                    xt_full[:, bg * B_SH : (bg + 1) * B_SH]
                    .reshape(NIT, P, N_PH, PH)
                    .transpose(2, 1, 0, 3)
                    .reshape(N_PH * P, NIT * PH)
                ),
                "wv": np.ascontiguousarray(
                    wv_f16[:, og * O_SH : (og + 1) * O_SH]
                ),
            }
        )
    return in_maps


def assemble_output(results):
    y_full = np.empty((B_FULL, O_FULL), dtype=np.float32)
    for core in range(N_CORES):
        bg, og = core // OG, core % OG
        y_full[bg * B_SH : (bg + 1) * B_SH, og * O_SH : (og + 1) * O_SH] = (
            np.asarray(results[core]["yT"], dtype=np.float32).T
        )
    return y_full


def kernel(x, cheby_coeffs):
    nc = get_nc()
    in_maps = prep_inputs(x, cheby_coeffs)
    res = run_bass_kernel_spmd(nc, in_maps, list(range(N_CORES)))
    return assemble_output(res.results)



# revision 10
# speedup vs baseline: 1.4070x; 1.4001x over previous
"""ChebyKAN layer kernel for Trainium2 (8 NeuronCores) — v2.

Computes y[b,o] = sum_{i,d} T_d(tanh(x)[b,i]) * C[i,o,d] as a
(8192 x 8320) @ (8320 x 1024) f16 matmul after building product-basis
features on-chip (see prep_inputs for the basis change; the constant
feature is folded into a 128-row ones-tile block).

Sharding: 4-way batch x 2-way output columns (core c -> batch group
c//2, output group c%2). No collectives.

Key layout decisions (all HW-probed, see session notes):
- Weights are the STATIONARY matmul operand; features stream as moving
  data. Each stationary [128i x 128o] slice serves 2 consecutive N=512
  matmuls (batch phase = 1024), which amortizes the otherwise
  serialized LDWEIGHTS (~280 -> ~225 ns/matmul measured). Explicit
  nc.tensor.ldweights is WORSE (double-emits the load).
- Output is produced transposed ([O_SH, B_SH] per core) and fixed on
  host, so no on-chip transposes are needed.
- Whole pipeline in fp16 (e5m10): x is transported as f16, the f16
  feature-chain intermediates feed the PE directly (no bf16 copies),
  f16 weights carry 2 extra mantissa bits over bf16. PSUM is f32.
  absmax-rel vs the f32 reference: 1.7e-3.
- x is host-packed so each phase loads one [128, 8192] f16 tile in a
  single DMA, and tanh runs once per phase as a single wide ACT op:
  per-tile DMA-gated producer ops measurably starve the PE (~3.3 us
  per gated tile-op), so the only x-gated op is 1 wide tanh per phase.
- PSUM evacuation runs on DVE (gpsimd cannot access PSUM) and the y
  store DMAs issue from the gpsimd ring, keeping the ACT stream (tanh/
  Square chain) clear of phase-boundary drains. The evac block for
  phase N is emitted after phase N+1's first feature-chain group (with
  the psum alloc + ones matmuls deferred past it, keeping pool
  generations in emission order), so the in-order DVE stream never
  queues next-phase chain work behind psum drains (+2% measured).
"""

from contextlib import ExitStack

import numpy as np

import concourse.bacc as bacc
import concourse.mybir as mybir
import concourse.tile as tile
from concourse.bass_utils import run_bass_kernel_spmd

P = 128
B_FULL, I_DIM, O_FULL, DEG = 8192, 1024, 1024, 8
N_CORES = 8
BG, OG = 4, 2  # core grid: batch groups x output groups
B_SH = B_FULL // BG  # 2048 batch rows per core
O_SH = O_FULL // OG  # 512 output cols per core
PH = 1024  # batch-phase width
N_PH = B_SH // PH  # 2 phases
NBH = PH // 512  # 2 moving-chunks per stationary
NIT = I_DIM // P  # 8 i-tiles
KT = 1 + DEG * NIT  # 65 contraction tiles (1 ones + 64 features)
NOT = O_SH // P  # 4 output tiles
WS = 64.0  # host-side weight scale (descaled in psum evac)
F32 = mybir.dt.float32
F16 = mybir.dt.float16
MULT = mybir.AluOpType.mult
ADD = mybir.AluOpType.add
TANH = mybir.ActivationFunctionType.Tanh
SQUARE = mybir.ActivationFunctionType.Square
COPY = mybir.ActivationFunctionType.Copy

USE_LDW = False  # probe: explicit ldweights double-emits the load (slower)

_NC_CACHE = []


def _build_ir(repeat=1, loop_iters=None, variant="full"):
    nc = bacc.Bacc(
        "TRN2", target_bir_lowering=False, debug=False, enable_asserts=False
    )
    xP = nc.dram_tensor("xP", [N_PH * P, NIT * PH], F16, kind="ExternalInput").ap()
    wv = nc.dram_tensor("wv", [KT * P, O_SH], F16, kind="ExternalInput").ap()
    yT = nc.dram_tensor("yT", [O_SH, B_SH], F32, kind="ExternalOutput").ap()

    with ExitStack() as ctx:
        tc = ctx.enter_context(tile.TileContext(nc))
        wpool = ctx.enter_context(tc.tile_pool(name="w", bufs=1))
        opool = ctx.enter_context(tc.tile_pool(name="o", bufs=1))
        xpool = ctx.enter_context(tc.tile_pool(name="x", bufs=2))
        fb = {"full4": 4, "full3": 3, "fullp": 3}.get(variant, 2)
        fpool = ctx.enter_context(tc.tile_pool(name="f", bufs=fb))
        cpool = ctx.enter_context(tc.tile_pool(name="c", bufs=fb))
        tpool = ctx.enter_context(tc.tile_pool(name="t1", bufs=2))
        ypool = ctx.enter_context(tc.tile_pool(name="yp", bufs=8))
        pspool = ctx.enter_context(tc.tile_pool(name="ps", bufs=8, space="PSUM"))

        ones = opool.tile([P, 512], F16, tag="ones")
        nc.vector.memset(ones[:], 1.0)

        wt = [
            wpool.tile([P, O_SH], F16, tag=f"w{k}", name=f"w{k}") for k in range(KT)
        ]
        nc.sync.dma_start(out=wt[0][:], in_=wv[0:P, :])
        if loop_iters is not None:
            # timing variants: weights fully resident before the hw loop
            for k in range(1, KT):
                nc.sync.dma_start(out=wt[k][:], in_=wv[k * P : (k + 1) * P, :])

        fixed_feat = None
        if variant in ("pe", "mm", "nochain", "tanhonly", "halfchain", "tanh1",
                       "chainload"):
            pfpool = ctx.enter_context(tc.tile_pool(name="pf", bufs=1))
            fixed_feat = [
                pfpool.tile([P, PH], F16, tag=f"pf{d}", name=f"pf{d}")
                for d in range(1, DEG + 1)
            ]
            for t in fixed_feat:
                nc.vector.memset(t[:], 0.01)
        xz = None
        if variant == "nox":
            xzpool = ctx.enter_context(tc.tile_pool(name="xz", bufs=1))
            xz = xzpool.tile([P, PH], F16, tag="xz")
            nc.vector.memset(xz[:], 0.3)

        def emit_body(rep):
            pending_evac = [None]

            def flush_evac():
                if pending_evac[0] is not None:
                    pending_evac[0]()
                    pending_evac[0] = None

            for ph in range(N_PH):
                fprev = [None]
                b0 = ph * PH
                xbig = None
                t1big = None
                if variant not in ("pe", "mm", "nox"):
                    xbig = xpool.tile([P, NIT * PH], F16, tag="xr")
                    nc.sync.dma_start(
                        out=xbig[:], in_=xP[ph * P : (ph + 1) * P, :]
                    )
                    # one wide tanh per phase: the only x-DMA-gated op; all
                    # downstream per-it work gates on this early single op
                    t1big = tpool.tile([P, NIT * PH], F16, tag="t1big")
                    nc.scalar.activation(t1big[:], xbig[:], TANH)
                def alloc_and_ones():
                    # psum alloc + ones k-tile; deferred to after flush_evac so
                    # pool generations stay in emission order
                    ps = [
                        [
                            pspool.tile([P, 512], F32, tag="ps", name="ps")
                            for _ in range(NBH)
                        ]
                        for _ in range(NOT)
                    ]
                    for ot in range(NOT):
                        s = wt[0][:, ot * P : (ot + 1) * P]
                        for bh in range(NBH):
                            nc.tensor.matmul(
                                ps[ot][bh][:], s, ones[:], start=True, stop=False
                            )
                    return ps

                psums = None
                for it in range(NIT):
                    if variant in ("pe", "mm", "nochain", "tanhonly", "halfchain", "tanh1"):
                        if psums is None:
                            flush_evac()
                            psums = alloc_and_ones()
                        if variant in ("tanhonly", "halfchain") or (
                            variant == "tanh1" and it == 0
                        ):
                            xr = xbig[:, it * PH : (it + 1) * PH]
                            f = [None] + list(fixed_feat)
                            ftmp = fpool.tile([P, PH], F16, tag="f1", name="f1")
                            nc.scalar.activation(ftmp[:], xr[:], TANH)
                            f[1] = ftmp
                            if variant == "halfchain":
                                f2t = fpool.tile([P, PH], F16, tag="f2", name="f2")
                                nc.scalar.activation(f2t[:], f[1][:], SQUARE)
                                f[2] = f2t
                                t2 = cpool.tile([P, PH], F16, tag="t2")
                                nc.vector.tensor_scalar(t2[:], f[2][:], 2.0, -1.0, MULT, ADD)
                                f3t = fpool.tile([P, PH], F16, tag="f3", name="f3")
                                nc.vector.tensor_tensor(f3t[:], f[1][:], t2[:], MULT)
                                f[3] = f3t
                                f4t = fpool.tile([P, PH], F16, tag="f4", name="f4")
                                nc.vector.tensor_tensor(f4t[:], t2[:], t2[:], MULT)
                                f[4] = f4t
                            for d in range(1, DEG + 1):
                                k = 1 + (d - 1) * NIT + it
                                last = it == NIT - 1 and d == DEG
                                for ot in range(NOT):
                                    sw = wt[k][:, ot * P : (ot + 1) * P]
                                    for bh in range(NBH):
                                        nc.tensor.matmul(
                                            psums[ot][bh][:],
                                            sw,
                                            f[d][:, bh * 512 : (bh + 1) * 512],
                                            start=False,
                                            stop=last,
                                        )
                            continue
                        f = [None] + fixed_feat
                        for d in range(1, DEG + 1):
                            k = 1 + (d - 1) * NIT + it
                            last = it == NIT - 1 and d == DEG
                            for ot in range(NOT):
                                s = wt[k][:, ot * P : (ot + 1) * P]
                                for bh in range(NBH):
                                    nc.tensor.matmul(
                                        psums[ot][bh][:],
                                        s,
                                        f[d][:, bh * 512 : (bh + 1) * 512],
                                        start=False,
                                        stop=last,
                                    )
                        continue
                    xr = xz if variant == "nox" else None
                    if ph == 0 and rep == 0 and loop_iters is None:
                        for d in range(1, DEG + 1):
                            k = 1 + (d - 1) * NIT + it
                            nc.sync.dma_start(
                                out=wt[k][:], in_=wv[k * P : (k + 1) * P, :]
                            )
                    # f16 feature chain; Chebyshev-product basis:
                    # f1=t, f2=t^2, t2=2f2-1, f3=t*t2, f4=t2^2, t4=2f4-1,
                    # f5=t*t4, f6=t2*t4, f7=t4*f3, f8=t4^2
                    f = [None] * (DEG + 1)

                    def feat(d):
                        f[d] = fpool.tile(
                            [P, PH], F16, tag=f"f{d}", name=f"f{d}"
                        )
                        return f[d]

                    f[1] = t1big[:, it * PH : (it + 1) * PH] if t1big is not None else None
                    if f[1] is None:
                        nc.scalar.activation(feat(1)[:], xr[:], TANH)
                    nc.scalar.activation(feat(2)[:], f[1][:], SQUARE)
                    t2 = cpool.tile([P, PH], F16, tag="t2")
                    nc.vector.tensor_scalar(t2[:], f[2][:], 2.0, -1.0, MULT, ADD)
                    nc.vector.tensor_tensor(feat(3)[:], f[1][:], t2[:], MULT)
                    nc.vector.tensor_tensor(feat(4)[:], t2[:], t2[:], MULT)
                    t4 = cpool.tile([P, PH], F16, tag="t4")
                    nc.scalar.activation(t4[:], f[4][:], COPY, bias=-1.0, scale=2.0)
                    e56 = nc.gpsimd if variant == "fullb" else nc.vector
                    e56.tensor_tensor(feat(5)[:], f[1][:], t4[:], MULT)
                    e56.tensor_tensor(feat(6)[:], t2[:], t4[:], MULT)
                    nc.vector.tensor_tensor(feat(7)[:], t4[:], f[3][:], MULT)
                    nc.vector.tensor_tensor(feat(8)[:], t4[:], t4[:], MULT)
                    if it == 0:
                        flush_evac()
                        psums = alloc_and_ones()

                    for d in range(1, DEG + 1):
                        k = 1 + (d - 1) * NIT + it
                        last = it == NIT - 1 and d == DEG
                        fd = (
                            fixed_feat[d - 1]
                            if variant == "chainload"
                            else f[d]
                        )
                        for ot in range(NOT):
                            s = wt[k][:, ot * P : (ot + 1) * P]
                            if USE_LDW:
                                nc.tensor.ldweights(s)
                            for bh in range(NBH):
                                nc.tensor.matmul(
                                    psums[ot][bh][:],
                                    s,
                                    fd[:, bh * 512 : (bh + 1) * 512],
                                    start=False,
                                    stop=last,
                                )
                def make_evac(psums, b0):
                    def emit():
                        if variant == "mm":
                            return
                        for ot in range(NOT):
                            for bh in range(NBH):
                                ysb = ypool.tile([P, 512], F32, tag="ysb")
                                if variant == "fullb":
                                    # evac on ACT: keeps the DVE queue free
                                    # for chain ops (JIT producer regime)
                                    nc.scalar.activation(
                                        ysb[:], psums[ot][bh][:], COPY,
                                        scale=1.0 / WS,
                                    )
                                else:
                                    # evac on DVE (gpsimd cannot touch PSUM)
                                    nc.vector.tensor_scalar(
                                        ysb[:], psums[ot][bh][:], 1.0 / WS, 0.0,
                                        MULT, ADD,
                                    )
                                if variant != "noy":
                                    nc.gpsimd.dma_start(
                                        out=yT[
                                            ot * P : (ot + 1) * P,
                                            b0 + bh * 512 : b0 + (bh + 1) * 512,
                                        ],
                                        in_=ysb[:],
                                    )
                    return emit

                flush_evac()  # no-op except variants that skip the it-loop
                pending_evac[0] = make_evac(psums, b0)
            flush_evac()

        if loop_iters is not None:
            with tc.For_i(0, loop_iters, 1):
                emit_body(0)
        else:
            for rep in range(repeat):
                emit_body(rep)
    nc.compile()
    return nc


def get_nc():
    if not _NC_CACHE:
        _NC_CACHE.append(_build_ir())
    return _NC_CACHE[0]


def prep_inputs(x, cheby_coeffs):
    """Host-side shard prep: returns per-core input maps."""
    x = np.asarray(x, dtype=np.float32)
    c = np.asarray(cheby_coeffs, dtype=np.float64)
    w = [c[:, :, d] for d in range(DEG + 1)]
    v = [
        w[0] - w[2] - w[4] + w[6] - w[8],
        w[1] - w[3] + w[5] - w[7],
        2.0 * (w[2] - w[6]),
        2.0 * (w[3] - w[5]),
        2.0 * w[4],
        2.0 * (w[5] - w[7]),
        2.0 * w[6],
        4.0 * w[7],
        2.0 * w[8],
    ]
    v0r = v[0].reshape(NIT, P, O_FULL).sum(axis=0)  # (128, 1024)
    wv_full = np.concatenate([v0r] + v[1:], axis=0) * WS  # (8320, 1024)
    wv_f16 = wv_full.astype(np.float16)
    xt_full = x.T.astype(np.float16)  # (1024, 8192)

    in_maps = []
    for core in range(N_CORES):
        bg, og = core // OG, core % OG
        in_maps.append(
            {
                # pack so each partition's per-phase x is contiguous:
                # xP[ph*128+p, it*PH+b] = xT[it*128+p, ph*PH+b]
                "xP": np.ascontiguousarray(
                    xt_full[:, bg * B_SH : (bg + 1) * B_SH]
                    .reshape(NIT, P, N_PH, PH)
                    .transpose(2, 1, 0, 3)
                    .reshape(N_PH * P, NIT * PH)
                ),
                "wv": np.ascontiguousarray(
                    wv_f16[:, og * O_SH : (og + 1) * O_SH]
                ),
            }
        )
    return in_maps


def assemble_output(results):
    y_full = np.empty((B_FULL, O_FULL), dtype=np.float32)
    for core in range(N_CORES):
        bg, og = core // OG, core % OG
        y_full[bg * B_SH : (bg + 1) * B_SH, og * O_SH : (og + 1) * O_SH] = (
            np.asarray(results[core]["yT"], dtype=np.float32).T
        )
    return y_full


def kernel(x, cheby_coeffs):
    nc = get_nc()
    in_maps = prep_inputs(x, cheby_coeffs)
    res = run_bass_kernel_spmd(nc, in_maps, list(range(N_CORES)))
    return assemble_output(res.results)

